# revision 48
# baseline (speedup 1.0000x reference)
"""Bass/Trainium2 kernel for nn_Attentioncell (Bahdanau-style attention cell).

Mathematical simplification (verified to rel-err ~6e-7 against the jax
reference): the per-step scores are
    scores[b,l] = (total[b,l,:] + (h @ W2)[b,:]) @ V
               = (total @ V)[b,l] + (h @ W2 @ V)[b]
and softmax over l is invariant to the per-b shift, so the attention
weights are identical for every timestep and independent of h:
    attn = softmax_l(x_static @ (W1 @ V))        (b2, W2, h0 drop out)
    ctx[b,:] = sum_l attn[b,l] * x_static[b,l,:]
    out[b,t,:] = x[b,t,:] @ W3[:D] + ctx[b,:] @ W3[D:] + b3

The scan disappears entirely; the kernel is a handful of matmuls and a
softmax, data-parallel over batch B=32 across 8 NeuronCores (4 per core).

Implementation notes:
  - compute dtype bf16 for everything TensorE/DVE-heavy (measured
    end-to-end rel err ~2.6e-3, well under the 2e-2 gate); exp and all
    reductions accumulate in f32.
  - all big inputs are host-permuted so each SBUF partition's data is
    one contiguous DRAM segment, and each tensor is a single dma_start:
    the first version used 2KB-segment DMAs on one queue and was
    dispatch-bound (~107 GB/s).
  - scores via DVE mul + ACT Copy/accum_out (per-partition sum);
    softmax normalizer Z and context are TensorE partition-reductions
    against a block-diagonal E = exp(scores)*mask.
  - out = sum_j xT_j^T @ W3top_j + Ind5^T @ [c2; b3] accumulated in one
    PSUM bank; Ind5 rows 0..3 are per-batch indicators over the 128
    (b,t) output rows, row 4 is ones (adds b3 to every row).
"""

import numpy as np

B, T, L, S, D = 32, 32, 196, 512, 512
NCORES = 8
BLOC = B // NCORES          # 4 batches per core
BT = BLOC * T               # 128 output rows per core
BL = BLOC * L               # 784 static rows per core
NCH = 7                     # bl chunks
CH = BL // NCH              # 112 rows per chunk

_cache = {}


def _build_graph():
    import concourse.bacc as bacc
    import concourse.tile as tile
    from concourse import mybir

    f32 = mybir.dt.float32
    bf16 = mybir.dt.bfloat16
    nc = bacc.Bacc("TRN2", target_bir_lowering=False, debug=False,
                   num_devices=NCORES)

    # xsp packs [w1vb | xs chunks 0..6 | mask] so every DMA group moves
    # >=2KB-per-partition segments (1KB segments are descriptor-bound).
    XSW = (NCH + 1) * S + NCH * BLOC
    xs_d = nc.dram_tensor("xsp", [CH, XSW], bf16, kind="ExternalInput").ap()
    # w3tx packs [xt slabs | w3t slabs]
    w3t_d = nc.dram_tensor("w3tx", [128, 4 * D + 512], bf16,
                           kind="ExternalInput").ap()
    w3b_d = nc.dram_tensor("w3b", [128, 4 * D], bf16, kind="ExternalInput").ap()
    b3_d = nc.dram_tensor("b3r", [1, D], bf16, kind="ExternalInput").ap()
    ind5_d = nc.dram_tensor("ind5", [5, BT], bf16, kind="ExternalInput").ap()
    id4_d = nc.dram_tensor("id4", [4, 4], bf16, kind="ExternalInput").ap()
    out_d = nc.dram_tensor("out", [BT, D], f32, kind="ExternalOutput").ap()

    with tile.TileContext(nc) as tc:
        with (
            tc.tile_pool(name="big", bufs=1) as big,
            tc.tile_pool(name="small", bufs=1) as small,
            tc.tile_pool(name="scratch", bufs=2) as scratch,
            tc.tile_pool(name="ps_acc", bufs=1, space="PSUM") as ps_acc,
            tc.tile_pool(name="ps_tr", bufs=2, space="PSUM") as ps_tr,
        ):
            xsp = big.tile([CH, XSW], bf16, tag="xsp")
            w1vb = xsp[:, 0:S]
            xs = xsp[:, S:(NCH + 1) * S]
            mask = xsp[:, (NCH + 1) * S:]
            w3tx = big.tile([128, 4 * D + 512], bf16, tag="w3tx")
            xt = w3tx[:, 0:512]
            w3t = w3tx[:, 512:]
            w3b = big.tile([128, 4 * D], bf16, tag="w3b")
            ind5 = small.tile([5, BT], bf16, tag="ind5")
            id4 = small.tile([4, 4], bf16, tag="id4")
            ones = small.tile([CH, 1], f32, tag="ones")
            scores = small.tile([CH, NCH], f32, tag="scores")
            etile = small.tile([CH, NCH], bf16, tag="etile")
            E = small.tile([CH, NCH * BLOC], bf16, tag="E")
            recipZ = small.tile([BLOC, 1], f32, tag="recipZ")
            ctx_sb = small.tile([BLOC, S], bf16, tag="ctx_sb")
            ctxT = small.tile([128, 4 * BLOC], bf16, tag="ctxT")
            rhs5 = small.tile([5, D], bf16, tag="rhs5")
            out_sb = big.tile([BT, D], f32, tag="out_sb")

            # ---- DMA loads. Each issuing engine owns one HW queue
            # (~150 GB/s each), so spread xs quarters across all four
            # queues to land the scores inputs as early as possible;
            # later-needed tensors queue up behind them. ----
            # DMA schedule: every queue ships its xs group first (scores
            # are the critical chain), then xt/W3top pieces (out-matmuls
            # run mid-kernel), then W3bot (needed last, for c2).
            nc.sync.dma_start(xsp[:, 0:3 * S], xs_d[:, 0:3 * S])
            nc.scalar.dma_start(xsp[:, 3 * S:5 * S], xs_d[:, 3 * S:5 * S])
            nc.gpsimd.dma_start(xsp[:, 5 * S:], xs_d[:, 5 * S:])
            nc.scalar.dma_start(w3tx[:, 0:3 * 512], w3t_d[:, 0:3 * 512])
            nc.gpsimd.dma_start(w3tx[:, 3 * 512:], w3t_d[:, 3 * 512:])
            nc.sync.dma_start(w3b[:], w3b_d[:])
            nc.scalar.dma_start(ind5[:], ind5_d[:])
            nc.scalar.dma_start(id4[:], id4_d[:])
            nc.scalar.dma_start(rhs5[4:5, :], b3_d[:])
            nc.vector.memset(ones[:], 1.0)
            out_ps = ps_acc.tile([BT, D], f32, tag="out_ps")

            # ---- scores[bl] = xs[bl,:] . w1v ----
            # (mul on DVE; per-partition sum alternates between ACT
            # Copy+accum_out and DVE tensor_reduce so neither engine
            # serializes the whole phase. tensor_tensor_reduce would
            # fuse this but wedges the DVE on this runtime.)
            # exp / E-build / ctx-matmul are split into two halves so
            # the PE starts accumulating ctx while the second half of
            # scores is still being computed.
            ctx_ps = ps_acc.tile([BLOC, S], f32, tag="ctx_ps")
            z_ps = ps_acc.tile([BLOC, 1], f32, tag="z_ps")

            w1vb_pair = w1vb.rearrange(
                "p (one s) -> p one s", one=1).to_broadcast((CH, 2, S))

            def scores_pair(c):
                # one DVE mul covers chunks c,c+1 (same DMA group, so no
                # extra wait); ACT reduces the even chunk, DVE the odd.
                prod = scratch.tile([CH, 2 * S], bf16, tag="prod")
                nc.vector.tensor_mul(
                    prod[:].rearrange("p (c s) -> p c s", c=2),
                    xs[:, c * S:(c + 2) * S].rearrange(
                        "p (c s) -> p c s", c=2),
                    w1vb_pair)
                dump = scratch.tile([CH, S], bf16, tag="dump")
                nc.scalar.activation(dump[:], prod[:, 0:S],
                                     mybir.ActivationFunctionType.Copy,
                                     accum_out=scores[:, c:c + 1])
                nc.vector.tensor_reduce(scores[:, c + 1:c + 2],
                                        prod[:, S:2 * S],
                                        axis=mybir.AxisListType.X,
                                        op=mybir.AluOpType.add)

            def scores_chunk(c):
                prod = scratch.tile([CH, S], bf16, tag="prods")
                nc.vector.tensor_mul(prod[:], xs[:, c * S:(c + 1) * S],
                                     w1vb[:])
                dump = scratch.tile([CH, S], bf16, tag="dumps")
                nc.scalar.activation(dump[:], prod[:],
                                     mybir.ActivationFunctionType.Copy,
                                     accum_out=scores[:, c:c + 1])

            def softmax_half(c0, c1):
                # E[:, 4c+b] = exp(scores[:,c]) * mask for c in [c0,c1)
                n = c1 - c0
                nc.scalar.activation(etile[:, c0:c1], scores[:, c0:c1],
                                     mybir.ActivationFunctionType.Exp)
                nc.vector.tensor_mul(
                    E[:, c0 * BLOC:c1 * BLOC].rearrange(
                        "p (c b) -> p c b", b=BLOC),
                    etile[:, c0:c1].to_broadcast((CH, n, BLOC)),
                    mask[:, c0 * BLOC:c1 * BLOC].rearrange(
                        "p (c b) -> p c b", b=BLOC),
                )
                for c in range(c0, c1):
                    nc.tensor.matmul(ctx_ps[:], E[:, c * BLOC:(c + 1) * BLOC],
                                     xs[:, c * S:(c + 1) * S],
                                     start=(c == 0), stop=(c == NCH - 1))

            scores_pair(0)
            scores_pair(2)
            softmax_half(0, 4)
            scores_pair(4)
            scores_chunk(6)
            softmax_half(4, NCH)

            # ---- out partial: x @ W3top (emitted after the ctx chain so
            # the PE prioritizes ctx; accumulation order is irrelevant) ----
            for j in range(4):
                nc.tensor.matmul(out_ps[:], xt[:, j * 128:(j + 1) * 128],
                                 w3t[:, j * D:(j + 1) * D],
                                 start=(j == 0), stop=False,
                                 skip_group_check=True)

            # Z: pre-sum E over chunks on DVE (strided view puts c
            # innermost), then a single [112,4]^T @ ones matmul.
            esum = small.tile([CH, BLOC], f32, tag="esum")
            nc.vector.tensor_reduce(
                esum[:],
                E[:].rearrange("p (c b) -> p b c", b=BLOC),
                axis=mybir.AxisListType.X,
                op=mybir.AluOpType.add)
            nc.tensor.matmul(z_ps[:], esum[:], ones[:], start=True, stop=True)
            nc.vector.reciprocal(recipZ[:], z_ps[:])
            nc.scalar.copy(ctx_sb[:], ctx_ps[:])

            # ---- transpose ctx ([4,512] -> 4x [128,4]) on PE ----
            for j in range(4):
                tr = ps_tr.tile([128, BLOC], bf16, tag="tr")
                nc.tensor.transpose(tr[:], ctx_sb[:, j * 128:(j + 1) * 128],
                                    id4[:])
                nc.vector.tensor_copy(ctxT[:, j * BLOC:(j + 1) * BLOC], tr[:])

            # ---- c2 = ctx @ W3bot (unnormalized), then scale by 1/Z ----
            c2_ps = ps_acc.tile([BLOC, D], f32, tag="c2_ps")
            for j in range(4):
                nc.tensor.matmul(c2_ps[:], ctxT[:, j * BLOC:(j + 1) * BLOC],
                                 w3b[:, j * D:(j + 1) * D],
                                 start=(j == 0), stop=(j == 3))
            nc.vector.tensor_scalar_mul(rhs5[0:4, :], c2_ps[:], recipZ[:])

            # ---- out += Ind5^T @ [c2; b3], in two row halves so the
            # copy-out + DMA of half 0 overlaps the matmul of half 1
            # (row split keeps the out DMA at 2KB/partition segments) ----
            H = BT // 2
            for h in range(2):
                sl = slice(h * H, (h + 1) * H)
                nc.tensor.matmul(out_ps[sl, :], ind5[:, sl], rhs5[:],
                                 start=False, stop=(h == 1),
                                 skip_group_check=True)
                if h == 0:
                    nc.scalar.copy(out_sb[sl, :], out_ps[sl, :])
                    nc.sync.dma_start(out_d[sl, :], out_sb[sl, :])
                else:
                    nc.vector.tensor_copy(out_sb[sl, :], out_ps[sl, :])
                    nc.scalar.dma_start(out_d[sl, :], out_sb[sl, :])

    nc.compile()
    return nc


def _get_graph():
    if "nc" not in _cache:
        _cache["nc"] = _build_graph()
    return _cache["nc"]


def _consts():
    if "consts" in _cache:
        return _cache["consts"]
    import ml_dtypes
    bf = ml_dtypes.bfloat16
    ind5 = np.zeros((5, BT), np.float32)
    for b in range(BLOC):
        ind5[b, b * T:(b + 1) * T] = 1.0
    ind5[4, :] = 1.0
    mask = np.zeros((CH, NCH, BLOC), np.float32)
    for c in range(NCH):
        for p in range(CH):
            mask[p, c, (c * CH + p) // L] = 1.0
    consts = {
        "ind5": np.ascontiguousarray(ind5.astype(bf)),
        "id4": np.ascontiguousarray(np.eye(4).astype(bf)),
        "_mask": mask.reshape(CH, NCH * BLOC).astype(np.float32),
    }
    _cache["consts"] = consts
    return consts


def kernel(x, x_static, h0, W1, W2, W3, b2, b3, V, **_unused):
    import ml_dtypes
    from concourse.bass_utils import run_bass_kernel_spmd
    bf = ml_dtypes.bfloat16

    x = np.asarray(x, np.float32)
    x_static = np.asarray(x_static, np.float32)
    W1 = np.asarray(W1, np.float32)
    W3 = np.asarray(W3, np.float32)
    b3 = np.asarray(b3, np.float32)
    V = np.asarray(V, np.float32)

    # Host-side weight folding (weights are per-model constants).
    w1v = (W1 @ V).reshape(-1).astype(np.float32)           # [S]
    w1vb = np.broadcast_to(w1v, (CH, S))
    # per-partition-contiguous permuted layouts (one big DMA segment
    # per partition):
    w3t = (W3[:D].reshape(4, 128, D).transpose(1, 0, 2)
           .reshape(128, 4 * D))
    w3b = np.ascontiguousarray(
        W3[D:].reshape(4, 128, D).transpose(1, 0, 2).reshape(128, 4 * D)
        .astype(bf))
    b3r = np.ascontiguousarray(b3.reshape(1, D).astype(bf))
    consts = _consts()

    nc = _get_graph()
    in_maps = []
    for i in range(NCORES):
        sl = slice(i * BLOC, (i + 1) * BLOC)
        xs_l = x_static[sl].reshape(BL, S)
        xs_p = xs_l.reshape(NCH, CH, S).transpose(1, 0, 2).reshape(CH, NCH * S)
        xsp = np.ascontiguousarray(
            np.concatenate([w1vb, xs_p, consts["_mask"]], axis=1).astype(bf))
        xt_l = x[sl].reshape(BT, D).T                        # [512, 128]
        xt_p = (xt_l.reshape(4, 128, 128).transpose(1, 0, 2)
                .reshape(128, 512))
        w3tx = np.ascontiguousarray(
            np.concatenate([xt_p, w3t], axis=1).astype(bf))
        in_maps.append({
            "xsp": xsp, "w3tx": w3tx,
            "w3b": w3b, "b3r": b3r,
            "ind5": consts["ind5"], "id4": consts["id4"],
        })
    res = run_bass_kernel_spmd(nc, in_maps, core_ids=list(range(NCORES)))
    out = np.empty((B, T, D), np.float32)
    for i in range(NCORES):
        out[i * BLOC:(i + 1) * BLOC] = res.results[i]["out"].reshape(BLOC, T, D)
    return out


# revision 50
# speedup vs baseline: 1.0284x; 1.0284x over previous
"""Bass/Trainium2 kernel for nn_Attentioncell (Bahdanau-style attention cell).

Mathematical simplification (verified to rel-err ~6e-7 against the jax
reference): the per-step scores are
    scores[b,l] = (total[b,l,:] + (h @ W2)[b,:]) @ V
               = (total @ V)[b,l] + (h @ W2 @ V)[b]
and softmax over l is invariant to the per-b shift, so the attention
weights are identical for every timestep and independent of h:
    attn = softmax_l(x_static @ (W1 @ V))        (b2, W2, h0 drop out)
    ctx[b,:] = sum_l attn[b,l] * x_static[b,l,:]
    out[b,t,:] = x[b,t,:] @ W3[:D] + ctx[b,:] @ W3[D:] + b3

The scan disappears entirely; the kernel is a handful of matmuls and a
softmax, data-parallel over batch B=32 across 8 NeuronCores (4 per core).

Implementation notes:
  - compute dtype bf16 for everything TensorE/DVE-heavy (measured
    end-to-end rel err ~2.6e-3, well under the 2e-2 gate); exp and all
    reductions accumulate in f32.
  - all big inputs are host-permuted so each SBUF partition's data is
    one contiguous DRAM segment, and each tensor is a single dma_start:
    the first version used 2KB-segment DMAs on one queue and was
    dispatch-bound (~107 GB/s).
  - scores via DVE mul + ACT Copy/accum_out (per-partition sum);
    softmax normalizer Z and context are TensorE partition-reductions
    against a block-diagonal E = exp(scores)*mask.
  - out = sum_j xT_j^T @ W3top_j + Ind5^T @ [c2; b3] accumulated in one
    PSUM bank; Ind5 rows 0..3 are per-batch indicators over the 128
    (b,t) output rows, row 4 is ones (adds b3 to every row).
"""

import numpy as np

B, T, L, S, D = 32, 32, 196, 512, 512
NCORES = 8
BLOC = B // NCORES          # 4 batches per core
BT = BLOC * T               # 128 output rows per core
BL = BLOC * L               # 784 static rows per core
NCH = 7                     # bl chunks
CH = BL // NCH              # 112 rows per chunk

_cache = {}


def _build_graph():
    import concourse.bacc as bacc
    import concourse.tile as tile
    from concourse import mybir

    f32 = mybir.dt.float32
    bf16 = mybir.dt.bfloat16
    nc = bacc.Bacc("TRN2", target_bir_lowering=False, debug=False,
                   num_devices=NCORES)

    # xsp packs [w1vb | xs chunks 0..6 | mask] so every DMA group moves
    # >=2KB-per-partition segments (1KB segments are descriptor-bound).
    XSW = (NCH + 1) * S + NCH * BLOC
    xs_d = nc.dram_tensor("xsp", [CH, XSW], bf16, kind="ExternalInput").ap()
    # w3tx packs [xt slabs | w3t slabs]
    w3t_d = nc.dram_tensor("w3tx", [128, 4 * D + 512], bf16,
                           kind="ExternalInput").ap()
    w3b_d = nc.dram_tensor("w3b", [128, 4 * D], bf16, kind="ExternalInput").ap()
    b3_d = nc.dram_tensor("b3r", [1, D], bf16, kind="ExternalInput").ap()
    ind5_d = nc.dram_tensor("ind5", [5, BT], bf16, kind="ExternalInput").ap()
    id4_d = nc.dram_tensor("id4", [4, 4], bf16, kind="ExternalInput").ap()
    out_d = nc.dram_tensor("out", [BT, D], f32, kind="ExternalOutput").ap()

    with tile.TileContext(nc) as tc:
        with (
            tc.tile_pool(name="big", bufs=1) as big,
            tc.tile_pool(name="small", bufs=1) as small,
            tc.tile_pool(name="scratch", bufs=2) as scratch,
            tc.tile_pool(name="ps_acc", bufs=1, space="PSUM") as ps_acc,
            tc.tile_pool(name="ps_tr", bufs=2, space="PSUM") as ps_tr,
        ):
            xsp = big.tile([CH, XSW], bf16, tag="xsp")
            w1vb = xsp[:, 0:S]
            xs = xsp[:, S:(NCH + 1) * S]
            mask = xsp[:, (NCH + 1) * S:]
            w3tx = big.tile([128, 4 * D + 512], bf16, tag="w3tx")
            xt = w3tx[:, 0:512]
            w3t = w3tx[:, 512:]
            w3b = big.tile([128, 4 * D], bf16, tag="w3b")
            ind5 = small.tile([5, BT], bf16, tag="ind5")
            id4 = small.tile([4, 4], bf16, tag="id4")
            ones = small.tile([CH, 1], f32, tag="ones")
            scores = small.tile([CH, NCH], f32, tag="scores")
            etile = small.tile([CH, NCH], bf16, tag="etile")
            E = small.tile([CH, NCH * BLOC], bf16, tag="E")
            recipZ = small.tile([BLOC, 1], f32, tag="recipZ")
            ctx_sb = small.tile([BLOC, S], bf16, tag="ctx_sb")
            ctxT = small.tile([128, 4 * BLOC], bf16, tag="ctxT")
            rhs5 = small.tile([5, D], bf16, tag="rhs5")
            out_sb = big.tile([BT, D], f32, tag="out_sb")

            # ---- DMA loads. Each issuing engine owns one HW queue
            # (~150 GB/s each), so spread xs quarters across all four
            # queues to land the scores inputs as early as possible;
            # later-needed tensors queue up behind them. ----
            # DMA schedule: every queue ships its xs group first (scores
            # are the critical chain), then xt/W3top pieces (out-matmuls
            # run mid-kernel), then W3bot (needed last, for c2).
            nc.sync.dma_start(xsp[:, 0:3 * S], xs_d[:, 0:3 * S])
            nc.scalar.dma_start(xsp[:, 3 * S:5 * S], xs_d[:, 3 * S:5 * S])
            nc.gpsimd.dma_start(xsp[:, 5 * S:], xs_d[:, 5 * S:])
            nc.scalar.dma_start(w3tx[:, 0:3 * 512], w3t_d[:, 0:3 * 512])
            nc.gpsimd.dma_start(w3tx[:, 3 * 512:], w3t_d[:, 3 * 512:])
            nc.sync.dma_start(w3b[:], w3b_d[:])
            nc.scalar.dma_start(ind5[:], ind5_d[:])
            nc.scalar.dma_start(id4[:], id4_d[:])
            nc.scalar.dma_start(rhs5[4:5, :], b3_d[:])
            nc.vector.memset(ones[:], 1.0)
            out_ps = ps_acc.tile([BT, D], f32, tag="out_ps")

            # ---- scores[bl] = xs[bl,:] . w1v ----
            # (mul on DVE; per-partition sum alternates between ACT
            # Copy+accum_out and DVE tensor_reduce so neither engine
            # serializes the whole phase. tensor_tensor_reduce would
            # fuse this but wedges the DVE on this runtime.)
            # exp / E-build / ctx-matmul are split into two halves so
            # the PE starts accumulating ctx while the second half of
            # scores is still being computed.
            ctx_ps = ps_acc.tile([BLOC, S], f32, tag="ctx_ps")
            z_ps = ps_acc.tile([BLOC, 1], f32, tag="z_ps")

            def scores_chunk(c):
                prod = scratch.tile([CH, S], bf16, tag="prod")
                nc.vector.tensor_mul(prod[:], xs[:, c * S:(c + 1) * S],
                                     w1vb[:])
                if c % 2 == 0:
                    dump = scratch.tile([CH, S], bf16, tag="dump")
                    nc.scalar.activation(dump[:], prod[:],
                                         mybir.ActivationFunctionType.Copy,
                                         accum_out=scores[:, c:c + 1])
                else:
                    nc.vector.tensor_reduce(scores[:, c:c + 1], prod[:],
                                            axis=mybir.AxisListType.X,
                                            op=mybir.AluOpType.add)

            def softmax_half(c0, c1):
                # E[:, 4c+b] = exp(scores[:,c]) * mask for c in [c0,c1)
                n = c1 - c0
                nc.scalar.activation(etile[:, c0:c1], scores[:, c0:c1],
                                     mybir.ActivationFunctionType.Exp)
                nc.vector.tensor_mul(
                    E[:, c0 * BLOC:c1 * BLOC].rearrange(
                        "p (c b) -> p c b", b=BLOC),
                    etile[:, c0:c1].to_broadcast((CH, n, BLOC)),
                    mask[:, c0 * BLOC:c1 * BLOC].rearrange(
                        "p (c b) -> p c b", b=BLOC),
                )
                for c in range(c0, c1):
                    nc.tensor.matmul(ctx_ps[:], E[:, c * BLOC:(c + 1) * BLOC],
                                     xs[:, c * S:(c + 1) * S],
                                     start=(c == 0), stop=(c == NCH - 1))

            for c in range(4):
                scores_chunk(c)
            softmax_half(0, 4)
            for c in range(4, NCH):
                scores_chunk(c)
            softmax_half(4, NCH)

            # ---- out partial: x @ W3top (emitted after the ctx chain so
            # the PE prioritizes ctx; accumulation order is irrelevant) ----
            for j in range(4):
                nc.tensor.matmul(out_ps[:], xt[:, j * 128:(j + 1) * 128],
                                 w3t[:, j * D:(j + 1) * D],
                                 start=(j == 0), stop=False,
                                 skip_group_check=True)

            # Z: pre-sum E over chunks on DVE (strided view puts c
            # innermost), then a single [112,4]^T @ ones matmul.
            esum = small.tile([CH, BLOC], f32, tag="esum")
            nc.vector.tensor_reduce(
                esum[:],
                E[:].rearrange("p (c b) -> p b c", b=BLOC),
                axis=mybir.AxisListType.X,
                op=mybir.AluOpType.add)
            nc.tensor.matmul(z_ps[:], esum[:], ones[:], start=True, stop=True)
            nc.vector.reciprocal(recipZ[:], z_ps[:])
            nc.scalar.copy(ctx_sb[:], ctx_ps[:])

            # ---- transpose ctx ([4,512] -> 4x [128,4]) on PE ----
            for j in range(4):
                tr = ps_tr.tile([128, BLOC], bf16, tag="tr")
                nc.tensor.transpose(tr[:], ctx_sb[:, j * 128:(j + 1) * 128],
                                    id4[:])
                nc.vector.tensor_copy(ctxT[:, j * BLOC:(j + 1) * BLOC], tr[:])

            # ---- c2 = ctx @ W3bot (unnormalized), then scale by 1/Z ----
            c2_ps = ps_acc.tile([BLOC, D], f32, tag="c2_ps")
            for j in range(4):
                nc.tensor.matmul(c2_ps[:], ctxT[:, j * BLOC:(j + 1) * BLOC],
                                 w3b[:, j * D:(j + 1) * D],
                                 start=(j == 0), stop=(j == 3))
            nc.vector.tensor_scalar_mul(rhs5[0:4, :], c2_ps[:], recipZ[:])

            # ---- out += Ind5^T @ [c2; b3], in two row halves so the
            # copy-out + DMA of half 0 overlaps the matmul of half 1
            # (row split keeps the out DMA at 2KB/partition segments) ----
            H = BT // 2
            for h in range(2):
                sl = slice(h * H, (h + 1) * H)
                nc.tensor.matmul(out_ps[sl, :], ind5[:, sl], rhs5[:],
                                 start=False, stop=(h == 1),
                                 skip_group_check=True)
                if h == 0:
                    nc.scalar.copy(out_sb[sl, :], out_ps[sl, :])
                    nc.sync.dma_start(out_d[sl, :], out_sb[sl, :])
                else:
                    nc.vector.tensor_copy(out_sb[sl, :], out_ps[sl, :])
                    nc.scalar.dma_start(out_d[sl, :], out_sb[sl, :])

    nc.compile()
    return nc


def _get_graph():
    if "nc" not in _cache:
        _cache["nc"] = _build_graph()
    return _cache["nc"]


def _consts():
    if "consts" in _cache:
        return _cache["consts"]
    import ml_dtypes
    bf = ml_dtypes.bfloat16
    ind5 = np.zeros((5, BT), np.float32)
    for b in range(BLOC):
        ind5[b, b * T:(b + 1) * T] = 1.0
    ind5[4, :] = 1.0
    mask = np.zeros((CH, NCH, BLOC), np.float32)
    for c in range(NCH):
        for p in range(CH):
            mask[p, c, (c * CH + p) // L] = 1.0
    consts = {
        "ind5": np.ascontiguousarray(ind5.astype(bf)),
        "id4": np.ascontiguousarray(np.eye(4).astype(bf)),
        "_mask": mask.reshape(CH, NCH * BLOC).astype(np.float32),
    }
    _cache["consts"] = consts
    return consts


def kernel(x, x_static, h0, W1, W2, W3, b2, b3, V, **_unused):
    import ml_dtypes
    from concourse.bass_utils import run_bass_kernel_spmd
    bf = ml_dtypes.bfloat16

    x = np.asarray(x, np.float32)
    x_static = np.asarray(x_static, np.float32)
    W1 = np.asarray(W1, np.float32)
    W3 = np.asarray(W3, np.float32)
    b3 = np.asarray(b3, np.float32)
    V = np.asarray(V, np.float32)

    # Host-side weight folding (weights are per-model constants).
    w1v = (W1 @ V).reshape(-1).astype(np.float32)           # [S]
    w1vb = np.broadcast_to(w1v, (CH, S))
    # per-partition-contiguous permuted layouts (one big DMA segment
    # per partition):
    w3t = (W3[:D].reshape(4, 128, D).transpose(1, 0, 2)
           .reshape(128, 4 * D))
    w3b = np.ascontiguousarray(
        W3[D:].reshape(4, 128, D).transpose(1, 0, 2).reshape(128, 4 * D)
        .astype(bf))
    b3r = np.ascontiguousarray(b3.reshape(1, D).astype(bf))
    consts = _consts()

    nc = _get_graph()
    in_maps = []
    for i in range(NCORES):
        sl = slice(i * BLOC, (i + 1) * BLOC)
        xs_l = x_static[sl].reshape(BL, S)
        xs_p = xs_l.reshape(NCH, CH, S).transpose(1, 0, 2).reshape(CH, NCH * S)
        xsp = np.ascontiguousarray(
            np.concatenate([w1vb, xs_p, consts["_mask"]], axis=1).astype(bf))
        xt_l = x[sl].reshape(BT, D).T                        # [512, 128]
        xt_p = (xt_l.reshape(4, 128, 128).transpose(1, 0, 2)
                .reshape(128, 512))
        w3tx = np.ascontiguousarray(
            np.concatenate([xt_p, w3t], axis=1).astype(bf))
        in_maps.append({
            "xsp": xsp, "w3tx": w3tx,
            "w3b": w3b, "b3r": b3r,
            "ind5": consts["ind5"], "id4": consts["id4"],
        })
    res = run_bass_kernel_spmd(nc, in_maps, core_ids=list(range(NCORES)))
    out = np.empty((B, T, D), np.float32)
    for i in range(NCORES):
        out[i * BLOC:(i + 1) * BLOC] = res.results[i]["out"].reshape(BLOC, T, D)
    return out
